# revision 1
# baseline (speedup 1.0000x reference)
"""JTNN graph-encoder message passing on 8 Trainium2 NeuronCores.

Sharding: data-parallel over bonds (message updates) and atoms (readout),
message table replicated per core and refreshed by AllGather each depth
iteration. Bonds are sorted by graph-neighbor count so sentinel-skipped
gather columns cluster, minimizing indirect-DMA instruction count; the
constant tree-neighbor contribution is gathered once, multiplied by
W_h^T, and folded into binput. Host does all index remapping; the device
program is SPMD-identical across cores.
"""
import sys
sys.path.insert(0, "/opt/trn_rl_repo")
import numpy as np

H = 450
HP = 512            # H padded to 4 k-chunks of 128
DEPTH = 6
NT = 16000          # tree message rows
NB = 40000          # real bonds
NBP = 40960         # bonds padded to 128*8*40
PB = NBP // 8       # bonds per core = 5120
TB = PB // 128      # bond tiles per core = 40
NA = 20000
PA = 2560           # atoms per core incl pad (2500 + 60)
TA = PA // 128      # atom tiles per core = 20
AF = 35
BF = 40
MAXNB = 15
NROWS = NT + NBP    # valid message rows
SENT = NROWS        # sentinel -> dedicated zero row appended to the table
N_CORES = 8

_CACHE = {}


def _build_program(KG, KT):
    import os
    STAGE = int(os.environ.get("K_STAGE", "9"))
    ATOM = int(os.environ.get("K_ATOM", "9"))
    import concourse.bacc as bacc
    import concourse.bass as bass
    import concourse.mybir as mybir
    import concourse.tile as tile
    from concourse.masks import make_identity

    f32 = mybir.dt.float32
    i32 = mybir.dt.int32
    Relu = mybir.ActivationFunctionType.Relu
    nc = bacc.Bacc(target_bir_lowering=False, num_devices=N_CORES)

    tree_sh = nc.dram_tensor("tree_sh", [NT // 8, H], f32, kind="ExternalInput")
    fbondsT = nc.dram_tensor("fbondsT", [BF, PB], f32, kind="ExternalInput")
    fatomsT = nc.dram_tensor("fatomsT", [AF, PA], f32, kind="ExternalInput")
    W_iT = nc.dram_tensor("W_iT", [BF, H], f32, kind="ExternalInput")
    W_hT = nc.dram_tensor("W_hT", [HP, H], f32, kind="ExternalInput")
    W_o1T = nc.dram_tensor("W_o1T", [AF, H], f32, kind="ExternalInput")
    W_o2T = nc.dram_tensor("W_o2T", [HP, H], f32, kind="ExternalInput")
    b_oT = nc.dram_tensor("b_oT", [H, 1], f32, kind="ExternalInput")
    idxg = nc.dram_tensor("idxg", [128, TB, MAXNB], i32, kind="ExternalInput")
    idxt = nc.dram_tensor("idxt", [128, TB, MAXNB], i32, kind="ExternalInput")
    idxa = nc.dram_tensor("idxa", [128, TA, MAXNB], i32, kind="ExternalInput")
    out_ms = nc.dram_tensor("out_ms", [4, 128, 128], f32, kind="ExternalOutput")

    RG = [list(range(N_CORES))]

    with tile.TileContext(nc) as tc:
        with (
            tc.tile_pool(name="const", bufs=1) as cp,
            tc.tile_pool(name="sbuf", bufs=3) as sp,
            tc.tile_pool(name="lhs", bufs=2) as lp,
            tc.tile_pool(name="acc", bufs=8) as accp,
            tc.tile_pool(name="neit", bufs=8) as ntp,
            tc.tile_pool(name="psum", bufs=4, space="PSUM") as pp,
            tc.tile_pool(name="apsum", bufs=2, space="PSUM") as app,
            tc.tile_pool(name="ypsum", bufs=2, space="PSUM") as ypp,
            tc.tile_pool(name="dram", bufs=1, space="DRAM") as dp,
        ):
            if os.environ.get("K_SHARED", "0") == "1":
                table = nc.dram_tensor("table", [NROWS + 1, H], f32,
                                       kind="Internal", addr_space="Shared")
            else:
                table = dp.tile([NROWS + 1, H], f32)
            y_loc = dp.tile([PB, H], f32)
            tree_b = dp.tile([NT // 8, H], f32)

            ident = cp.tile([128, 128], f32)
            make_identity(nc, ident[:])
            w_it = cp.tile([BF, H], f32)
            nc.sync.dma_start(w_it[:], W_iT[:])
            w_ht = cp.tile([128, 4, H], f32)
            nc.sync.dma_start(w_ht[:], W_hT[:].rearrange("(c k) h -> k c h", k=128))
            w_o1t = cp.tile([AF, H], f32)
            nc.sync.dma_start(w_o1t[:], W_o1T[:])
            w_o2t = cp.tile([128, 4, H], f32)
            nc.sync.dma_start(w_o2t[:], W_o2T[:].rearrange("(c k) h -> k c h", k=128))
            b_ot = cp.tile([128, 4], f32)
            nc.vector.memset(b_ot[:], 0.0)
            nc.sync.dma_start(b_ot[:, 0:3],
                              b_oT[0:384, :].rearrange("(c k) o -> k (c o)", k=128))
            nc.sync.dma_start(b_ot[0:66, 3:4], b_oT[384:450, :])
            ig = cp.tile([128, TB, MAXNB], i32)
            nc.sync.dma_start(ig[:], idxg[:])
            it_ = cp.tile([128, TB, MAXNB], i32)
            nc.sync.dma_start(it_[:], idxt[:])
            ia = cp.tile([128, TA, MAXNB], i32)
            nc.sync.dma_start(ia[:], idxa[:])
            fat = cp.tile([AF, PA], f32)
            nc.sync.dma_start(fat[:], fatomsT[:])
            binput = cp.tile([128, TB, H], f32)

            # zero row for sentinel gathers
            zr = sp.tile([128, H], f32, tag="y")
            nc.vector.memset(zr[:], 0.0)
            nc.sync.dma_start(table[NROWS:NROWS + 1, :], zr[:1, :])
            # tree shard -> bounce -> AllGather into table[0:NT]
            nc.gpsimd.dma_start(tree_b[:], tree_sh[:])
            nc.gpsimd.collective_compute(
                "AllGather", mybir.AluOpType.bypass, replica_groups=RG,
                ins=[tree_b[:]], outs=[table[0:NT, :]])

            # binput = fbonds @ W_i.T ; y0 = relu(binput)
            for t in range(TB if STAGE >= 2 else 0):
                fb = lp.tile([BF, 128], f32, tag="fb")
                nc.sync.dma_start(fb[:], fbondsT[:, t * 128:(t + 1) * 128])
                ps = ypp.tile([128, H], f32, tag="yps")
                nc.tensor.matmul(ps[:], lhsT=fb[:], rhs=w_it[:],
                                 start=True, stop=True)
                nc.vector.tensor_copy(binput[:, t, :], ps[:])
                y0 = sp.tile([128, H], f32, tag="y")
                nc.scalar.activation(y0[:], ps[:], Relu)
                nc.sync.dma_start(y_loc[t * 128:(t + 1) * 128, :], y0[:])
            if STAGE >= 2:
                nc.gpsimd.collective_compute(
                    "AllGather", mybir.AluOpType.bypass, replica_groups=RG,
                    ins=[y_loc[:]], outs=[table[NT:NROWS, :]])

            def gather_sum(idx_tile, t, K):
                acc = accp.tile([128, H], f32, tag="acc")
                nc.vector.memset(acc[:], 0.0)
                for k in range(K):
                    nc.gpsimd.indirect_dma_start(
                        out=acc[:], out_offset=None, in_=table[:],
                        in_offset=bass.IndirectOffsetOnAxis(
                            ap=idx_tile[:, t, k:k + 1], axis=0),
                        compute_op=mybir.AluOpType.add)
                return acc

            def nei_chunks(acc, tag):
                nts = []
                for c in range(4):
                    w = min(128, H - c * 128)
                    tp = pp.tile([128, 128], f32, tag="tp")
                    nc.tensor.transpose(tp[:w, :], acc[:, c * 128:c * 128 + w],
                                        ident[:])
                    nt_ = ntp.tile([128, 128], f32, tag=tag)
                    if w < 128:
                        nc.vector.memset(nt_[:], 0.0)
                    nc.vector.tensor_copy(nt_[:w, :], tp[:w, :])
                    nts.append(nt_)
                return nts

            def transform(acc):
                """psum_y [128,H] = (sum-gathered acc) @ W_h.T"""
                psy = ypp.tile([128, H], f32, tag="yps")
                nts = nei_chunks(acc, "ntb")
                for c in range(4):
                    nc.tensor.matmul(psy[:], lhsT=nts[c][:], rhs=w_ht[:, c, :],
                                     start=(c == 0), stop=(c == 3))
                return psy

            # fold constant tree contribution into binput
            for t in range(TB if STAGE >= 3 else 0):
                if KT[t] == 0:
                    continue
                psy = transform(gather_sum(it_, t, KT[t]))
                nc.vector.tensor_add(binput[:, t, :], binput[:, t, :], psy[:])

            # DEPTH-1 message update iterations
            n_iters = (DEPTH - 1 if STAGE >= 5 else (1 if STAGE >= 4 else 0))
            n_iters = int(os.environ.get("K_ITERS", n_iters)) if STAGE >= 5 else n_iters
            for i in range(n_iters):
                for t in range(TB):
                    psy = transform(gather_sum(ig, t, KG[t]))
                    y = sp.tile([128, H], f32, tag="y")
                    nc.vector.tensor_add(y[:], binput[:, t, :], psy[:])
                    nc.scalar.activation(y[:], y[:], Relu)
                    nc.sync.dma_start(y_loc[t * 128:(t + 1) * 128, :], y[:])
                nc.gpsimd.collective_compute(
                    "AllGather", mybir.AluOpType.bypass, replica_groups=RG,
                    ins=[y_loc[:]], outs=[table[NT:NROWS, :]])

            # atom readout: ahT[j, atom] = relu(W_o @ [fatoms; nei] + b_o)
            ahT = cp.tile([128, 4, PA], f32)
            nc.vector.memset(ahT[:], 0.0)
            for t in range(TA if STAGE >= 6 else 0):
                acc = gather_sum(ia, t, MAXNB)
                if ATOM < 2:
                    nc.vector.tensor_copy(ahT[:, 0, t * 128:(t + 1) * 128],
                                          acc[:, 0:128])
                    continue
                nts = nei_chunks(acc, "nta")
                if ATOM < 3:
                    nc.vector.tensor_copy(ahT[:, 0, t * 128:(t + 1) * 128],
                                          nts[0][:])
                    continue
                for j in range(4):
                    jw = min(128, H - j * 128)
                    ps = app.tile([128, 128], f32, tag="aps")
                    nc.tensor.matmul(ps[:jw, :],
                                     lhsT=w_o1t[:, j * 128:j * 128 + jw],
                                     rhs=fat[:, t * 128:(t + 1) * 128],
                                     start=True, stop=False)
                    for c in range(4):
                        nc.tensor.matmul(ps[:jw, :],
                                         lhsT=w_o2t[:, c, j * 128:j * 128 + jw],
                                         rhs=nts[c][:], start=False, stop=(c == 3))
                    nc.scalar.activation(ahT[:jw, j, t * 128:(t + 1) * 128],
                                         ps[:jw, :], Relu,
                                         bias=b_ot[:jw, j:j + 1])
            # segment sums over 20-atom molecules -> [128, 128] per j-chunk
            for j in range(4):
                red = sp.tile([128, 128], f32, tag="red")
                nc.vector.tensor_reduce(
                    red[:], ahT[:, j, :].rearrange("p (m a) -> p m a", a=20),
                    axis=mybir.AxisListType.X, op=mybir.AluOpType.add)
                nc.sync.dma_start(out_ms[j], red[:])

    nc.compile()
    return nc


def _prep(inputs):
    fatoms = np.asarray(inputs["fatoms"], np.float32)
    fbonds = np.asarray(inputs["fbonds"], np.float32)
    agraph = np.asarray(inputs["agraph"], np.int32)
    bgraph = np.asarray(inputs["bgraph"], np.int32)
    mol_ids = np.asarray(inputs["mol_ids"], np.int32)
    n_mols = int(inputs["n_mols"])
    tree = np.asarray(inputs["tree_message"], np.float32)
    W_i = np.asarray(inputs["W_i"], np.float32)
    W_h = np.asarray(inputs["W_h"], np.float32)
    W_o = np.asarray(inputs["W_o"], np.float32)
    b_o = np.asarray(inputs["b_o"], np.float32)

    n_graph = (bgraph >= NT).sum(1)
    order = np.argsort(-n_graph, kind="stable")
    order_p = np.concatenate([order, np.arange(NB, NBP)])
    gt = np.arange(NBP) // 128          # global tile of sorted position
    core_of = gt % 8
    ltile = gt // 8
    slot = np.arange(NBP) % 128
    trow = NT + core_of * PB + ltile * 128 + slot
    pos_of_bond = np.empty(NBP, np.int64)
    pos_of_bond[order_p] = np.arange(NBP)
    row_of_bond = trow[pos_of_bond]

    def remap(idx):
        out = idx.astype(np.int64).copy()
        g = out >= NT
        out[g] = row_of_bond[out[g] - NT]
        return out

    bg_r = remap(bgraph)
    ag_r = remap(agraph)

    # per-bond slots reordered: graph indices first, then tree, sentinel pad
    isg = bgraph >= NT
    srt = np.argsort(~isg, axis=1, kind="stable")        # graph cols first
    bg_sorted_slots = np.take_along_axis(bg_r, srt, axis=1)
    ngb = n_graph.astype(np.int64)
    colpos = np.arange(MAXNB)[None, :]
    idxg_all = np.where(colpos < ngb[:, None], bg_sorted_slots, SENT)
    tree_first = np.take_along_axis(bg_r, np.argsort(isg, axis=1, kind="stable"),
                                    axis=1)
    idxt_all = np.where(colpos < (MAXNB - ngb)[:, None], tree_first, SENT)
    idxg_all = np.concatenate([idxg_all, np.full((NBP - NB, MAXNB), SENT)])
    idxt_all = np.concatenate([idxt_all, np.full((NBP - NB, MAXNB), SENT)])

    ng_p = np.concatenate([n_graph[order], np.zeros(NBP - NB, np.int64)])
    nt_p = np.concatenate([MAXNB - n_graph[order], np.zeros(NBP - NB, np.int64)])
    KG = [int(ng_p[ltile == lt].max()) for lt in range(TB)]
    KT = [int(nt_p[ltile == lt].max()) for lt in range(TB)]

    idxg_sorted = idxg_all[order_p]
    idxt_sorted = idxt_all[order_p]
    fb_p = np.concatenate([fbonds, np.zeros((NBP - NB, BF), np.float32)])
    fb_sorted = fb_p[order_p]
    W_o1T = np.ascontiguousarray(W_o[:, :AF].T)
    W_o2T = np.zeros((HP, H), np.float32)
    W_o2T[:H] = W_o[:, AF:].T
    W_hTp = np.zeros((HP, H), np.float32)
    W_hTp[:H] = W_h.T
    counts = np.bincount(mol_ids, minlength=n_mols).astype(np.float32)

    in_maps = []
    for c in range(8):
        csel = np.nonzero(core_of == c)[0]               # sorted positions
        o = np.lexsort((slot[csel], ltile[csel]))
        p_idx = csel[o]                                  # (ltile, slot) order
        ig_c = idxg_sorted[p_idx].reshape(TB, 128, MAXNB).transpose(1, 0, 2)
        it_c = idxt_sorted[p_idx].reshape(TB, 128, MAXNB).transpose(1, 0, 2)
        a0 = c * (NA // 8)
        ag_c = np.full((PA, MAXNB), SENT, np.int64)
        ag_c[:NA // 8] = ag_r[a0:a0 + NA // 8]
        ag_c = ag_c.reshape(TA, 128, MAXNB).transpose(1, 0, 2)
        fat_c = np.zeros((AF, PA), np.float32)
        fat_c[:, :NA // 8] = fatoms[a0:a0 + NA // 8].T
        in_maps.append({
            "tree_sh": tree[c * (NT // 8):(c + 1) * (NT // 8)],
            "fbondsT": np.ascontiguousarray(fb_sorted[p_idx].T),
            "fatomsT": fat_c,
            "W_iT": np.ascontiguousarray(W_i.T),
            "W_hT": W_hTp,
            "W_o1T": W_o1T,
            "W_o2T": W_o2T,
            "b_oT": np.ascontiguousarray(b_o[:, None]),
            "idxg": np.minimum(ig_c, 2**31 - 1).astype(np.int32),
            "idxt": np.minimum(it_c, 2**31 - 1).astype(np.int32),
            "idxa": np.minimum(ag_c, 2**31 - 1).astype(np.int32),
        })
    return in_maps, KG, KT, counts


def get_program_and_maps(inputs):
    in_maps, KG, KT, counts = _prep(inputs)
    key = (tuple(KG), tuple(KT))
    if key not in _CACHE:
        _CACHE[key] = _build_program(KG, KT)
    return _CACHE[key], in_maps, counts


def postprocess(results, counts):
    outs = []
    for c in range(N_CORES):
        ms = np.asarray(results[c]["out_ms"])    # [4, 128(j), 128(mol)]
        msf = ms.reshape(HP, 128)[:H, :125]
        outs.append(msf.T)
    sums = np.concatenate(outs, axis=0)
    return (sums / counts[:, None]).astype(np.float32)


def kernel(**inputs) -> np.ndarray:
    from concourse.bass_utils import run_bass_kernel_spmd

    nc, in_maps, counts = get_program_and_maps(inputs)
    res = run_bass_kernel_spmd(nc, in_maps, core_ids=list(range(N_CORES)))
    return postprocess(res.results, counts)

